# revision 6
# baseline (speedup 1.0000x reference)
"""DeepSeekV3 router (moe_routing) Bass kernel for 8x TRN2 NeuronCores.

Strategy: data-parallel over tokens (T sharded 8 ways), kernel_DE/bias_E
replicated. Per core:
  - stream 16 token-tiles of 128 tokens
  - PE transpose x tiles (identity matmul) -> x^T chunks in SBUF
  - fp32r matmuls (N=256, full rate) accumulate z = x @ W in PSUM
  - ACT sigmoid -> scores
  - DVE routing: bias add, per-group top-2 via max8, top-4 group mask,
    masked top-8 via max8 + max_index, iota-match gather of unbiased
    scores, normalize, scale.
"""

import numpy as np

import concourse.bass as bass
import concourse.mybir as mybir
from concourse import bacc
from concourse.bass_utils import run_bass_kernel_spmd
from concourse.masks import make_identity
from concourse.tile import TileContext

F32 = mybir.dt.float32
F32R = mybir.dt.float32r
I32 = mybir.dt.int32
U32 = mybir.dt.uint32

T, D, E = 16384, 7168, 256
N_CORES = 8
TOP_K = 8
N_GROUPS = 8
TOPK_GROUPS = 4
EPG = E // N_GROUPS  # experts per group = 32
SCALE = 2.5

P = 128
TS = T // N_CORES          # tokens per core
KC = D // P                # contraction chunks = 56
TG = 8                     # transposes per PSUM staging buffer
ACT_COPIES = 4             # of the KC//TG=7 stage copies, how many go to ACT

# set by test harness to experiment; keep defaults for grading
TRANSPOSE_DTYPE = F32
MATMUL_DTYPE = F32


def build(ts: int = TS) -> bass.Bass:
    nt = ts // P  # token tiles per core
    nc = bacc.Bacc("TRN2", target_bir_lowering=False)

    x_dram = nc.dram_tensor("x", [ts, D], F32, kind="ExternalInput")
    w_dram = nc.dram_tensor("w", [D, E], F32, kind="ExternalInput")
    b_dram = nc.dram_tensor("bias", [E], F32, kind="ExternalInput")
    ow_dram = nc.dram_tensor("out_w", [ts, TOP_K], F32, kind="ExternalOutput")
    oi_dram = nc.dram_tensor("out_i", [ts, TOP_K], I32, kind="ExternalOutput")

    with TileContext(nc) as tc:
        with (
            tc.tile_pool(name="consts", bufs=1) as cp,
            tc.tile_pool(name="nat", bufs=2) as natp,
            tc.tile_pool(name="xt", bufs=2) as xtp,
            tc.tile_pool(name="stg", bufs=2, space=bass.MemorySpace.PSUM) as stgp,
            tc.tile_pool(name="zp", bufs=2, space=bass.MemorySpace.PSUM) as zpp,
            tc.tile_pool(name="sc", bufs=2) as scp,
            tc.tile_pool(name="rt", bufs=2) as rp,
            tc.tile_pool(name="outp", bufs=3) as op_,
        ):
            # ---- constants ----
            ident = cp.tile([P, P], F32)
            make_identity(nc, ident)

            bias_rep = cp.tile([P, E], F32)
            nc.gpsimd.dma_start(
                out=bias_rep,
                in_=bass.AP(tensor=b_dram, offset=0, ap=[[0, P], [1, E]]),
            )

            iota_i = cp.tile([P, E], I32)
            nc.gpsimd.iota(iota_i, pattern=[[1, E]], base=0, channel_multiplier=0)
            iota_f = cp.tile([P, E], F32)
            nc.vector.tensor_copy(iota_f, iota_i)

            # resident weights: w_sb[p, c, e] = W[c*128 + p, e]
            w_sb = cp.tile([P, KC, E], F32)
            w_re = w_dram.rearrange("(c p) e -> p c e", p=P)
            for wi in range(0, KC, 8):
                nc.sync.dma_start(
                    out=w_sb[:, wi : wi + 8, :], in_=w_re[:, wi : wi + 8, :]
                )

            nat_tiles: dict[int, object] = {}
            xt_tiles: dict[int, object] = {}

            def load_tile(i):
                nat = natp.tile([P, D], F32, tag="nat")
                nat_tiles[i] = nat
                nc.sync.dma_start(out=nat, in_=x_dram[i * P : (i + 1) * P, :])

            def transpose_tile(i):
                nat = nat_tiles.pop(i)
                xt = xtp.tile([P, D], F32, tag="xt")
                xt_tiles[i] = xt
                for g in range(KC // TG):
                    stage = stgp.tile([P, TG * P], TRANSPOSE_DTYPE, tag="stage")
                    for j in range(TG):
                        c = g * TG + j
                        nc.tensor.transpose(
                            stage[:, j * P : (j + 1) * P],
                            nat[:, c * P : (c + 1) * P].bitcast(TRANSPOSE_DTYPE),
                            ident.bitcast(TRANSPOSE_DTYPE),
                        )
                    dst = xt[:, g * TG * P : (g + 1) * TG * P]
                    if g < ACT_COPIES:
                        nc.scalar.copy(dst, stage.bitcast(F32))
                    else:
                        nc.vector.tensor_copy(dst, stage.bitcast(F32))

            def compute_tile(i):
                xt = xt_tiles.pop(i)
                z = zpp.tile([P, E], F32, tag="z")
                for c in range(KC):
                    nc.tensor.matmul(
                        z,
                        xt[:, c * P : (c + 1) * P].bitcast(MATMUL_DTYPE),
                        w_sb[:, c, :].bitcast(MATMUL_DTYPE),
                        start=(c == 0),
                        stop=(c == KC - 1),
                    )
                scores = scp.tile([P, E], F32, tag="scores")
                nc.scalar.activation(
                    scores, z, mybir.ActivationFunctionType.Sigmoid
                )

                # ---- routing ----
                biased = rp.tile([P, E], F32, tag="biased")
                nc.vector.tensor_add(biased, scores, bias_rep)

                gmax = rp.tile([P, N_GROUPS * 8], F32, tag="gmax")
                for g in range(N_GROUPS):
                    nc.vector.max(
                        gmax[:, g * 8 : (g + 1) * 8],
                        biased[:, g * EPG : (g + 1) * EPG],
                    )
                gm3 = gmax.rearrange("p (g k) -> p g k", k=8)
                gsc = rp.tile([P, N_GROUPS], F32, tag="gsc")
                gsc3 = gsc.rearrange("p (g k) -> p g k", k=1)
                nc.vector.tensor_add(gsc3, gm3[:, :, 0:1], gm3[:, :, 1:2])

                g8 = rp.tile([P, 8], F32, tag="g8")
                nc.vector.max(g8, gsc)
                maskg = rp.tile([P, N_GROUPS], F32, tag="maskg")
                nc.vector.tensor_scalar(
                    maskg,
                    gsc,
                    g8[:, TOPK_GROUPS - 1 : TOPK_GROUPS],
                    None,
                    op0=mybir.AluOpType.is_ge,
                )

                masked = rp.tile([P, E], F32, tag="masked")
                mg3 = maskg.rearrange("p (g k) -> p g k", k=1)
                nc.vector.tensor_tensor(
                    masked.rearrange("p (g e) -> p g e", g=N_GROUPS),
                    biased.rearrange("p (g e) -> p g e", g=N_GROUPS),
                    mg3.to_broadcast([P, N_GROUPS, EPG]),
                    op=mybir.AluOpType.mult,
                )

                top8 = rp.tile([P, 8], F32, tag="top8")
                nc.vector.max(top8, masked)
                idx = rp.tile([P, 8], U32, tag="idx")
                nc.vector.max_index(idx, top8, masked)
                idxf = rp.tile([P, 8], F32, tag="idxf")
                nc.vector.tensor_copy(idxf, idx)

                wg = rp.tile([P, 8], F32, tag="wg")
                scratch = rp.tile([P, E], F32, tag="scratch")
                for k in range(TOP_K):
                    nc.vector.scalar_tensor_tensor(
                        scratch,
                        iota_f,
                        idxf[:, k : k + 1],
                        scores,
                        op0=mybir.AluOpType.is_equal,
                        op1=mybir.AluOpType.mult,
                        accum_out=wg[:, k : k + 1],
                    )

                ssum = rp.tile([P, 1], F32, tag="ssum")
                nc.vector.tensor_reduce(
                    ssum, wg, axis=mybir.AxisListType.X, op=mybir.AluOpType.add
                )
                nc.vector.tensor_scalar_add(ssum, ssum, 1e-20)
                rinv = rp.tile([P, 1], F32, tag="rinv")
                nc.vector.reciprocal(rinv, ssum)
                nc.vector.tensor_scalar_mul(rinv, rinv, SCALE)

                wout = op_.tile([P, TOP_K], F32, tag="wout")
                nc.vector.tensor_tensor(
                    wout, wg, rinv.to_broadcast([P, TOP_K]), op=mybir.AluOpType.mult
                )
                iout = op_.tile([P, TOP_K], I32, tag="iout")
                nc.vector.tensor_copy(iout, idx)

                nc.sync.dma_start(
                    out=ow_dram[i * P : (i + 1) * P, :], in_=wout
                )
                nc.sync.dma_start(
                    out=oi_dram[i * P : (i + 1) * P, :], in_=iout
                )

            # ---- software-pipelined main loop ----
            load_tile(0)
            for i in range(nt + 1):
                if i + 1 < nt:
                    load_tile(i + 1)
                if i >= 1:
                    compute_tile(i - 1)
                if i < nt:
                    transpose_tile(i)

    nc.compile()
    return nc


def kernel(x_TD: np.ndarray, kernel_DE: np.ndarray, bias_E: np.ndarray):
    nc = build(TS)
    x_TD = np.ascontiguousarray(x_TD, dtype=np.float32)
    kernel_DE = np.ascontiguousarray(kernel_DE, dtype=np.float32)
    bias_E = np.ascontiguousarray(bias_E, dtype=np.float32)
    in_maps = [
        {
            "x": x_TD[c * TS : (c + 1) * TS],
            "w": kernel_DE,
            "bias": bias_E,
        }
        for c in range(N_CORES)
    ]
    res = run_bass_kernel_spmd(nc, in_maps, list(range(N_CORES)))
    w = np.concatenate([r["out_w"] for r in res.results], axis=0)
    i = np.concatenate([r["out_i"] for r in res.results], axis=0)
    return w.astype(np.float32), i.astype(np.int32)


# revision 9
# speedup vs baseline: 1.2363x; 1.2363x over previous
"""DeepSeekV3 router (moe_routing) Bass kernel for 8x TRN2 NeuronCores.

Strategy: data-parallel over tokens (T sharded 8 ways), kernel_DE/bias_E
replicated. Per core:
  - stream 16 token-tiles of 128 tokens
  - PE transpose x tiles (identity matmul) -> x^T chunks in SBUF
  - fp32r matmuls (N=256, full rate) accumulate z = x @ W in PSUM
  - ACT sigmoid -> scores
  - DVE routing: bias add, per-group top-2 via max8, top-4 group mask,
    masked top-8 via max8 + max_index, iota-match gather of unbiased
    scores, normalize, scale.
"""

import numpy as np

import concourse.bass as bass
import concourse.mybir as mybir
from concourse import bacc
from concourse.bass_utils import run_bass_kernel_spmd
from concourse.masks import make_identity
from concourse.tile import TileContext

F32 = mybir.dt.float32
F32R = mybir.dt.float32r
I32 = mybir.dt.int32
U32 = mybir.dt.uint32

T, D, E = 16384, 7168, 256
N_CORES = 8
TOP_K = 8
N_GROUPS = 8
TOPK_GROUPS = 4
EPG = E // N_GROUPS  # experts per group = 32
SCALE = 2.5

P = 128
TS = T // N_CORES          # tokens per core
KC = D // P                # contraction chunks = 56
TG = 8                     # transposes per PSUM staging buffer
ACT_COPIES = 4             # of the KC//TG=7 stage copies, how many go to ACT

# set by test harness to experiment; keep defaults for grading
TRANSPOSE_DTYPE = F32
MATMUL_DTYPE = F32


def build(ts: int = TS) -> bass.Bass:
    nt = ts // P  # token tiles per core
    nc = bacc.Bacc("TRN2", target_bir_lowering=False)

    x_dram = nc.dram_tensor("x", [ts, D], F32, kind="ExternalInput")
    w_dram = nc.dram_tensor("w", [D, E], F32, kind="ExternalInput")
    b_dram = nc.dram_tensor("bias", [E], F32, kind="ExternalInput")
    ow_dram = nc.dram_tensor("out_w", [ts, TOP_K], F32, kind="ExternalOutput")
    oi_dram = nc.dram_tensor("out_i", [ts, TOP_K], I32, kind="ExternalOutput")

    with TileContext(nc) as tc:
        with (
            tc.tile_pool(name="consts", bufs=1) as cp,
            tc.tile_pool(name="nat", bufs=2) as natp,
            tc.tile_pool(name="xt", bufs=2) as xtp,
            tc.tile_pool(name="stg", bufs=2, space=bass.MemorySpace.PSUM) as stgp,
            tc.tile_pool(name="zp", bufs=2, space=bass.MemorySpace.PSUM) as zpp,
            tc.tile_pool(name="sc", bufs=2) as scp,
            tc.tile_pool(name="rt", bufs=2) as rp,
            tc.tile_pool(name="outp", bufs=3) as op_,
        ):
            # ---- constants ----
            ident = cp.tile([P, P], F32)
            make_identity(nc, ident)

            bias_rep = cp.tile([P, E], F32)
            nc.gpsimd.dma_start(
                out=bias_rep,
                in_=bass.AP(tensor=b_dram, offset=0, ap=[[0, P], [1, E]]),
            )

            iota_i = cp.tile([P, E], I32)
            nc.gpsimd.iota(iota_i, pattern=[[1, E]], base=0, channel_multiplier=0)
            iota_f = cp.tile([P, E], F32)
            nc.vector.tensor_copy(iota_f, iota_i)

            nat_tiles: dict[int, object] = {}
            xt_tiles: dict[int, object] = {}
            z_tiles: dict[int, object] = {}

            def load_tile(i, split_first=False):
                nat = natp.tile([P, D], F32, tag="nat")
                nat_tiles[i] = nat
                rows = x_dram[i * P : (i + 1) * P, :]
                if split_first:
                    # let the first transposes start after ~1MB arrives
                    nc.sync.dma_start(out=nat[:, : TG * P], in_=rows[:, : TG * P])
                    nc.sync.dma_start(out=nat[:, TG * P :], in_=rows[:, TG * P :])
                else:
                    nc.sync.dma_start(out=nat, in_=rows)

            load_tile(0, split_first=True)

            # resident weights: w_sb[p, c, e] = W[c*128 + p, e]
            # (after the first x tile so the PE's first transposes aren't
            # stuck behind 7MB of weights on the DMA queue)
            w_sb = cp.tile([P, KC, E], F32)
            w_re = w_dram.rearrange("(c p) e -> p c e", p=P)
            for wi in range(0, KC, 8):
                nc.sync.dma_start(
                    out=w_sb[:, wi : wi + 8, :], in_=w_re[:, wi : wi + 8, :]
                )

            def transpose_group(i, g):
                nat = nat_tiles[i]
                if i not in xt_tiles:
                    xt_tiles[i] = xtp.tile([P, D], F32, tag="xt", name="xt")
                xt = xt_tiles[i]
                stage = stgp.tile([P, TG * P], TRANSPOSE_DTYPE, tag="stage")
                for j in range(TG):
                    c = g * TG + j
                    nc.tensor.transpose(
                        stage[:, j * P : (j + 1) * P],
                        nat[:, c * P : (c + 1) * P].bitcast(TRANSPOSE_DTYPE),
                        ident.bitcast(TRANSPOSE_DTYPE),
                    )
                dst = xt[:, g * TG * P : (g + 1) * TG * P]
                if g < ACT_COPIES:
                    nc.scalar.copy(dst, stage.bitcast(F32))
                else:
                    nc.vector.tensor_copy(dst, stage.bitcast(F32))
                if g == KC // TG - 1:
                    nat_tiles.pop(i)

            def matmul_group(i, g):
                xt = xt_tiles[i]
                if i not in z_tiles:
                    z_tiles[i] = zpp.tile([P, E], F32, tag="z", name="z")
                z = z_tiles[i]
                for c in range(g * TG, (g + 1) * TG):
                    nc.tensor.matmul(
                        z,
                        xt[:, c * P : (c + 1) * P].bitcast(MATMUL_DTYPE),
                        w_sb[:, c, :].bitcast(MATMUL_DTYPE),
                        start=(c == 0),
                        stop=(c == KC - 1),
                    )
                if g == KC // TG - 1:
                    xt_tiles.pop(i)

            def compute_tile(i):
                z = z_tiles.pop(i)
                scores = scp.tile([P, E], F32, tag="scores")
                nc.scalar.activation(
                    scores, z, mybir.ActivationFunctionType.Sigmoid
                )

                # ---- routing ----
                biased = rp.tile([P, E], F32, tag="biased")
                nc.vector.tensor_add(biased, scores, bias_rep)

                gmax = rp.tile([P, N_GROUPS * 8], F32, tag="gmax")
                for g in range(N_GROUPS):
                    nc.vector.max(
                        gmax[:, g * 8 : (g + 1) * 8],
                        biased[:, g * EPG : (g + 1) * EPG],
                    )
                gm3 = gmax.rearrange("p (g k) -> p g k", k=8)
                gsc = rp.tile([P, N_GROUPS], F32, tag="gsc")
                gsc3 = gsc.rearrange("p (g k) -> p g k", k=1)
                nc.vector.tensor_add(gsc3, gm3[:, :, 0:1], gm3[:, :, 1:2])

                g8 = rp.tile([P, 8], F32, tag="g8")
                nc.vector.max(g8, gsc)
                maskg = rp.tile([P, N_GROUPS], F32, tag="maskg")
                nc.vector.tensor_scalar(
                    maskg,
                    gsc,
                    g8[:, TOPK_GROUPS - 1 : TOPK_GROUPS],
                    None,
                    op0=mybir.AluOpType.is_ge,
                )

                masked = rp.tile([P, E], F32, tag="masked")
                mg3 = maskg.rearrange("p (g k) -> p g k", k=1)
                nc.vector.tensor_tensor(
                    masked.rearrange("p (g e) -> p g e", g=N_GROUPS),
                    biased.rearrange("p (g e) -> p g e", g=N_GROUPS),
                    mg3.to_broadcast([P, N_GROUPS, EPG]),
                    op=mybir.AluOpType.mult,
                )

                top8 = rp.tile([P, 8], F32, tag="top8")
                nc.vector.max(top8, masked)
                idx = rp.tile([P, 8], U32, tag="idx")
                nc.vector.max_index(idx, top8, masked)
                idxf = rp.tile([P, 8], F32, tag="idxf")
                nc.vector.tensor_copy(idxf, idx)

                wg = rp.tile([P, 8], F32, tag="wg")
                scratch = rp.tile([P, E], F32, tag="scratch")
                for k in range(TOP_K):
                    nc.vector.scalar_tensor_tensor(
                        scratch,
                        iota_f,
                        idxf[:, k : k + 1],
                        scores,
                        op0=mybir.AluOpType.is_equal,
                        op1=mybir.AluOpType.mult,
                        accum_out=wg[:, k : k + 1],
                    )

                ssum = rp.tile([P, 1], F32, tag="ssum")
                nc.vector.tensor_reduce(
                    ssum, wg, axis=mybir.AxisListType.X, op=mybir.AluOpType.add
                )
                nc.vector.tensor_scalar_add(ssum, ssum, 1e-20)
                rinv = rp.tile([P, 1], F32, tag="rinv")
                nc.vector.reciprocal(rinv, ssum)
                nc.vector.tensor_scalar_mul(rinv, rinv, SCALE)

                wout = op_.tile([P, TOP_K], F32, tag="wout")
                nc.vector.tensor_tensor(
                    wout, wg, rinv.to_broadcast([P, TOP_K]), op=mybir.AluOpType.mult
                )
                iout = op_.tile([P, TOP_K], I32, tag="iout")
                nc.vector.tensor_copy(iout, idx)

                nc.sync.dma_start(
                    out=ow_dram[i * P : (i + 1) * P, :], in_=wout
                )
                nc.sync.dma_start(
                    out=oi_dram[i * P : (i + 1) * P, :], in_=iout
                )

            # ---- software-pipelined main loop ----
            # software pipeline, interleaved at chunk-group granularity so the
            # PE can alternate between transposes (gated on x DMA) and matmuls
            # (gated on W DMA / copies) without FIFO head-of-line blocking
            for i in range(nt + 1):
                if i + 1 < nt:
                    load_tile(i + 1)
                for g in range(KC // TG):
                    if i < nt:
                        transpose_group(i, g)
                    if i >= 1:
                        matmul_group(i - 1, g)
                if i >= 1:
                    compute_tile(i - 1)

    nc.compile()
    return nc


def kernel(x_TD: np.ndarray, kernel_DE: np.ndarray, bias_E: np.ndarray):
    nc = build(TS)
    x_TD = np.ascontiguousarray(x_TD, dtype=np.float32)
    kernel_DE = np.ascontiguousarray(kernel_DE, dtype=np.float32)
    bias_E = np.ascontiguousarray(bias_E, dtype=np.float32)
    in_maps = [
        {
            "x": x_TD[c * TS : (c + 1) * TS],
            "w": kernel_DE,
            "bias": bias_E,
        }
        for c in range(N_CORES)
    ]
    res = run_bass_kernel_spmd(nc, in_maps, list(range(N_CORES)))
    w = np.concatenate([r["out_w"] for r in res.results], axis=0)
    i = np.concatenate([r["out_i"] for r in res.results], axis=0)
    return w.astype(np.float32), i.astype(np.int32)
